# revision 1
# baseline (speedup 1.0000x reference)
"""Trainium2 Bass kernel: gated linear recurrence encoder (B=64, T=2048, D=300).

Math (per example, torch Linear convention):
    z = tanh(x @ Wz.T + bz)
    o = tanh(x @ Wo.T + bo)
    c_t = g_t * c_{t-1} + (1 - g_t) * z_t     (c_{-1} = 0)
    h_t = o_t * c_t

Sharding: batch 64 -> 8 cores x 8 examples (data parallel); weights
replicated.  Device layout is feature-major [D, T] per example (host
pre-transposes), so matmuls produce z^T/o^T directly and the recurrence
runs as hardware tensor_tensor_scan instructions along the time axis.

Design points (all measured via pipelined-dispatch slope timing, 8 cores):
  - fp16 end-to-end: halves HBM traffic vs fp32 (IO is the v1 bottleneck:
    60 MB/core @ ~271 GB/s).  PSUM accumulation and the scan state remain
    f32, so rel err stays ~2e-3 (40 MB of fp16 traffic runs in ~67 us).
  - Matmul chunks MUST be 100 wide: K=M=100 runs at ~230 ns/instr while
    128-wide chunks run ~2.2x slower on this hardware (any dtype).
  - Wz/Wo are concatenated into one [300, 600] stationary, column order
    [z0|o0|z1|o2|z2|o2] in 100-wide chunks, so z_j / o_j / gate_j land on
    identical partitions for the elementwise stage (no realign copies).
  - The host also sends gm1 = g - 1, so d1 = gm1 * z is a plain
    tensor_tensor multiply (fp16 2x packed mode) instead of the 1x-only
    scalar_tensor_tensor; the scan c = g*c - d1 and h = o*c stay on the
    vector engine (Pool/GPSIMD is ~3x slower for these ops).
  - DMA is spread across the three DMA-capable queues (sync/scalar HWDGE,
    gpsimd SWDGE): x+gm1_0 on sync, g+gm1_1 on scalar, h+gm1_2 on gpsimd.
"""

import numpy as np

B, T, D = 64, 2048, 300
N_CORES = 8
BL = B // N_CORES      # examples per core
NT = 512               # matmul moving-dim tile
NNT = T // NT          # 4
CH = [(0, 100), (100, 200), (200, 300)]             # K / elementwise chunks
MCH = [(i * 100, (i + 1) * 100) for i in range(6)]  # combined z|o out chunks

_CACHE = {}
PROFILE = False
LAST_RESULTS = None


def _build_nc(bl=BL, repeat=1):
    import concourse.bacc as bacc
    import concourse.mybir as mybir
    from concourse import tile

    f32 = mybir.dt.float32
    f16 = mybir.dt.float16
    Alu = mybir.AluOpType
    ActF = mybir.ActivationFunctionType

    nc = bacc.Bacc()
    xT = nc.dram_tensor("xT", [bl, D, T], f16, kind="ExternalInput")
    gT = nc.dram_tensor("gT", [bl, D, T], f16, kind="ExternalInput")
    gm1T = nc.dram_tensor("gm1T", [bl, D, T], f16, kind="ExternalInput")
    Wcat = nc.dram_tensor("Wcat", [D, 600], f16, kind="ExternalInput")
    bcat = nc.dram_tensor("bcat", [600, 1], f32, kind="ExternalInput")
    hT = nc.dram_tensor("hT", [bl, D, T], f16, kind="ExternalOutput")

    with tile.TileContext(nc) as tc:
        with (
            tc.tile_pool(name="wpool", bufs=1) as wpool,
            tc.tile_pool(name="xpool", bufs=2) as xpool,
            tc.tile_pool(name="gpool", bufs=2) as gpool,
            tc.tile_pool(name="zpool", bufs=2) as zpool,
            tc.tile_pool(name="vpool", bufs=2) as vpool,
            tc.tile_pool(name="psum", bufs=6, space="PSUM") as psum,
        ):
            w_t, b_t = [], []
            for ki, (k0, k1) in enumerate(CH):
                wk = wpool.tile([k1 - k0, 600], f16, tag=f"w{ki}")
                nc.scalar.dma_start(wk[:], Wcat[k0:k1, :])
                w_t.append(wk)
            for mi, (m0, m1) in enumerate(MCH):
                bm = wpool.tile([m1 - m0, 1], f32, tag=f"b{mi}")
                nc.gpsimd.dma_start(bm[:], bcat[m0:m1, :])
                b_t.append(bm)

            for _rep in range(repeat):
                for b in range(bl):
                    xk, gk, gmk = [], [], []
                    gmq = [nc.sync, nc.scalar, nc.gpsimd]
                    for ki, (k0, k1) in enumerate(CH):
                        xkt = xpool.tile([k1 - k0, T], f16, tag=f"x{ki}",
                                         name=f"x{ki}")
                        nc.sync.dma_start(xkt[:], xT[b, k0:k1, :])
                        xk.append(xkt)
                        gkt = gpool.tile([k1 - k0, T], f16, tag=f"g{ki}",
                                         name=f"g{ki}")
                        nc.scalar.dma_start(gkt[:], gT[b, k0:k1, :])
                        gk.append(gkt)
                        gmt = gpool.tile([k1 - k0, T], f16, tag=f"gm{ki}",
                                         name=f"gm{ki}")
                        gmq[ki].dma_start(gmt[:], gm1T[b, k0:k1, :])
                        gmk.append(gmt)

                    zo = [
                        zpool.tile([m1 - m0, T], f16, tag=f"zo{mi}",
                                   name=f"zo{mi}")
                        for mi, (m0, m1) in enumerate(MCH)
                    ]
                    for n in range(NNT):
                        ns = slice(n * NT, (n + 1) * NT)
                        for mi, (m0, m1) in enumerate(MCH):
                            pm = psum.tile([m1 - m0, NT], f32,
                                           tag="pz", name="pz")
                            for ki in range(len(CH)):
                                nc.tensor.matmul(
                                    pm[:],
                                    w_t[ki][:, m0:m1],
                                    xk[ki][:, ns],
                                    start=(ki == 0),
                                    stop=(ki == len(CH) - 1),
                                )
                            nc.scalar.activation(
                                zo[mi][:, ns], pm[:], ActF.Tanh,
                                bias=b_t[mi][:]
                            )

                    for j in range(3):
                        gt = gk[j][:]
                        zt, ot = zo[2 * j][:], zo[2 * j + 1][:]
                        d1 = vpool.tile([100, T], f16, tag="d1", name="d1")
                        nc.vector.tensor_mul(d1[:], gmk[j][:], zt)
                        ct = vpool.tile([100, T], f16, tag="c", name="c")
                        nc.vector.tensor_tensor_scan(
                            ct[:], gt, d1[:], 0.0,
                            op0=Alu.mult, op1=Alu.subtract
                        )
                        ht = vpool.tile([100, T], f16, tag="h", name="h")
                        nc.vector.tensor_mul(ht[:], ot, ct[:])
                        k0, k1 = CH[j]
                        nc.gpsimd.dma_start(hT[b, k0:k1, :], ht[:])
    nc.compile()
    return nc


def _get_nc():
    if "nc" not in _CACHE:
        _CACHE["nc"] = _build_nc()
    return _CACHE["nc"]


def _make_in_maps(gate_encoding, inputs_encoding, Wz, bz, Wo, bo):
    gate_encoding = np.asarray(gate_encoding, dtype=np.float32)
    inputs_encoding = np.asarray(inputs_encoding, dtype=np.float32)
    WzT = np.asarray(Wz, dtype=np.float32).T   # [d_in, e_out]
    WoT = np.asarray(Wo, dtype=np.float32).T
    bz = np.asarray(bz, dtype=np.float32)
    bo = np.asarray(bo, dtype=np.float32)

    wparts, bparts = [], []
    for c0, c1 in CH:
        wparts += [WzT[:, c0:c1], WoT[:, c0:c1]]
        bparts += [bz[c0:c1], bo[c0:c1]]
    Wcat = np.ascontiguousarray(
        np.concatenate(wparts, axis=1).astype(np.float16))
    bcat = np.concatenate(bparts).reshape(600, 1).astype(np.float32)

    in_maps = []
    for c in range(N_CORES):
        sl = slice(c * BL, (c + 1) * BL)
        gTc = gate_encoding[sl].transpose(0, 2, 1)
        in_maps.append({
            "xT": np.ascontiguousarray(
                inputs_encoding[sl].transpose(0, 2, 1).astype(np.float16)),
            "gT": np.ascontiguousarray(gTc.astype(np.float16)),
            "gm1T": np.ascontiguousarray((gTc - 1.0).astype(np.float16)),
            "Wcat": Wcat,
            "bcat": bcat,
        })
    return in_maps


def kernel(gate_encoding, inputs_encoding, Wz, bz, Wo, bo):
    from concourse.bass_utils import run_bass_kernel_spmd

    nc = _get_nc()
    in_maps = _make_in_maps(gate_encoding, inputs_encoding, Wz, bz, Wo, bo)
    res = run_bass_kernel_spmd(nc, in_maps, list(range(N_CORES)),
                               trace=PROFILE)
    global LAST_RESULTS
    LAST_RESULTS = res

    hT_full = np.concatenate([r["hT"] for r in res.results], axis=0)
    return np.ascontiguousarray(
        hT_full.transpose(0, 2, 1).astype(np.float32))



# revision 2
# speedup vs baseline: 1.8586x; 1.8586x over previous
"""Trainium2 Bass kernel: gated linear recurrence encoder (B=64, T=2048, D=300).

v5: 6 m-chunks of 100 + wide PSUM (4 banks/chunk) + one wide [100,2048]
activation per chunk (ACT busy 117us -> 91us vs per-n-tile acts).

Matmul chunks MUST stay 100 wide: HW-measured M=120/124 chunks run ~1.5x
slower per instr (the cost model does not capture this cliff; measured via
interleaved single-dispatch min-diff: M=100 structure 116.1us vs the same
structure at M=120/124 157.8us).  Columns [z0|o0|z1|o1|z2|o2]; feature
chunks align 1:1 with m-chunks so no partition packing or shift copies are
needed.  PSUM: 2 tiles x 4 banks, double-buffered.  Queues: sync=x+g,
scalar=weights/bias (HWDGE), gpsimd=gm1+hT out (SWDGE) + offloaded h_0
multiply.  Elementwise per feature chunk on DVE: d1 = gm1*z (2x packed),
c = scan(g, d1) (1x), h = o*c (2x); h_0 on gpsimd to trim DVE (97.8us
busy vs PE ~107us real).
"""

import numpy as np

B, T, D = 64, 2048, 300
N_CORES = 8
BL = B // N_CORES
NT = 512
NNT = T // NT
CH = [(0, 100), (100, 200), (200, 300)]
MCH = [(i * 100, (i + 1) * 100) for i in range(6)]   # z0 o0 z1 o1 z2 o2
FCH = [(0, 100, 0, 1), (100, 200, 2, 3), (200, 300, 4, 5)]

OFFLOAD = True

_CACHE = {}
PROFILE = False
LAST_RESULTS = None


def _build_nc(bl=BL, repeat=1, offload=None):
    import concourse.bacc as bacc
    import concourse.mybir as mybir
    from concourse import tile

    if offload is None:
        offload = OFFLOAD
    f32 = mybir.dt.float32
    f16 = mybir.dt.float16
    Alu = mybir.AluOpType
    ActF = mybir.ActivationFunctionType

    nc = bacc.Bacc()
    xT = nc.dram_tensor("xT", [bl, D, T], f16, kind="ExternalInput")
    gT = nc.dram_tensor("gT", [bl, D, T], f16, kind="ExternalInput")
    gm1T = nc.dram_tensor("gm1T", [bl, D, T], f16, kind="ExternalInput")
    Wcat = nc.dram_tensor("Wcat", [D, 600], f16, kind="ExternalInput")
    bcat = nc.dram_tensor("bcat", [600, 1], f32, kind="ExternalInput")
    hT = nc.dram_tensor("hT", [bl, D, T], f16, kind="ExternalOutput")

    with tile.TileContext(nc) as tc:
        with (
            tc.tile_pool(name="wpool", bufs=1) as wpool,
            tc.tile_pool(name="xpool", bufs=2) as xpool,
            tc.tile_pool(name="gpool", bufs=2) as gpool,
            tc.tile_pool(name="zpool", bufs=2) as zpool,
            tc.tile_pool(name="vpool", bufs=2) as vpool,
            tc.tile_pool(name="psum", bufs=2, space="PSUM") as psum,
        ):
            w_t, b_t = [], []
            for ki, (k0, k1) in enumerate(CH):
                wk = wpool.tile([k1 - k0, 600], f16, tag=f"w{ki}")
                nc.scalar.dma_start(wk[:], Wcat[k0:k1, :])
                w_t.append(wk)
            for mi, (m0, m1) in enumerate(MCH):
                bm = wpool.tile([m1 - m0, 1], f32, tag=f"b{mi}")
                nc.scalar.dma_start(bm[:], bcat[m0:m1, :])
                b_t.append(bm)

            for _rep in range(repeat):
                for b in range(bl):
                    xk = []
                    for ki, (k0, k1) in enumerate(CH):
                        xkt = xpool.tile([k1 - k0, T], f16, tag=f"x{ki}",
                                         name=f"x{ki}")
                        nc.sync.dma_start(xkt[:], xT[b, k0:k1, :])
                        xk.append(xkt)
                    gk, gmk = [], []
                    for fi, (f0, f1, _, _) in enumerate(FCH):
                        gkt = gpool.tile([f1 - f0, T], f16, tag=f"g{fi}",
                                         name=f"g{fi}")
                        nc.sync.dma_start(gkt[:], gT[b, f0:f1, :])
                        gk.append(gkt)
                        gmt = gpool.tile([f1 - f0, T], f16, tag=f"gm{fi}",
                                         name=f"gm{fi}")
                        nc.gpsimd.dma_start(gmt[:], gm1T[b, f0:f1, :])
                        gmk.append(gmt)

                    zo = []
                    for mi, (m0, m1) in enumerate(MCH):
                        mw = m1 - m0
                        zot = zpool.tile([mw, T], f16, tag=f"zo{mi}",
                                         name=f"zo{mi}")
                        pm = psum.tile([mw, T], f32, tag="pz", name="pz")
                        for n in range(NNT):
                            ns = slice(n * NT, (n + 1) * NT)
                            for ki in range(len(CH)):
                                nc.tensor.matmul(
                                    pm[:, ns],
                                    w_t[ki][:, m0:m1],
                                    xk[ki][:, ns],
                                    start=(ki == 0),
                                    stop=(ki == len(CH) - 1),
                                )
                        nc.scalar.activation(
                            zot[:], pm[:], ActF.Tanh, bias=b_t[mi][:]
                        )
                        zo.append(zot)

                    for fi, (f0, f1, zi, oi) in enumerate(FCH):
                        fw = f1 - f0
                        d1 = vpool.tile([fw, T], f16, tag=f"d1_{fi}",
                                        name=f"d1_{fi}")
                        nc.vector.tensor_mul(d1[:], gmk[fi][:], zo[zi][:])
                        ct = vpool.tile([fw, T], f16, tag=f"c_{fi}",
                                        name=f"c_{fi}")
                        nc.vector.tensor_tensor_scan(
                            ct[:], gk[fi][:], d1[:], 0.0,
                            op0=Alu.mult, op1=Alu.subtract
                        )
                        ht = vpool.tile([fw, T], f16, tag=f"h_{fi}",
                                        name=f"h_{fi}")
                        eng = (nc.gpsimd if (offload and fi == 0)
                               else nc.vector)
                        eng.tensor_mul(ht[:], zo[oi][:], ct[:])
                        nc.gpsimd.dma_start(hT[b, f0:f1, :], ht[:])
    nc.compile()
    return nc


def _get_nc():
    if "nc" not in _CACHE:
        _CACHE["nc"] = _build_nc()
    return _CACHE["nc"]


def _make_in_maps(gate_encoding, inputs_encoding, Wz, bz, Wo, bo):
    gate_encoding = np.asarray(gate_encoding, dtype=np.float32)
    inputs_encoding = np.asarray(inputs_encoding, dtype=np.float32)
    WzT = np.asarray(Wz, dtype=np.float32).T
    WoT = np.asarray(Wo, dtype=np.float32).T
    bz = np.asarray(bz, dtype=np.float32)
    bo = np.asarray(bo, dtype=np.float32)

    wparts, bparts = [], []
    for c0, c1 in [(0, 100), (100, 200), (200, 300)]:
        wparts += [WzT[:, c0:c1], WoT[:, c0:c1]]
        bparts += [bz[c0:c1], bo[c0:c1]]
    Wcat = np.ascontiguousarray(
        np.concatenate(wparts, axis=1).astype(np.float16))
    bcat = np.concatenate(bparts).reshape(600, 1).astype(np.float32)

    in_maps = []
    for c in range(N_CORES):
        sl = slice(c * BL, (c + 1) * BL)
        gTc = gate_encoding[sl].transpose(0, 2, 1)
        in_maps.append({
            "xT": np.ascontiguousarray(
                inputs_encoding[sl].transpose(0, 2, 1).astype(np.float16)),
            "gT": np.ascontiguousarray(gTc.astype(np.float16)),
            "gm1T": np.ascontiguousarray((gTc - 1.0).astype(np.float16)),
            "Wcat": Wcat,
            "bcat": bcat,
        })
    return in_maps


def kernel(gate_encoding, inputs_encoding, Wz, bz, Wo, bo):
    from concourse.bass_utils import run_bass_kernel_spmd

    nc = _get_nc()
    in_maps = _make_in_maps(gate_encoding, inputs_encoding, Wz, bz, Wo, bo)
    res = run_bass_kernel_spmd(nc, in_maps, list(range(N_CORES)),
                               trace=PROFILE)
    global LAST_RESULTS
    LAST_RESULTS = res

    hT_full = np.concatenate([r["hT"] for r in res.results], axis=0)
    return np.ascontiguousarray(
        hT_full.transpose(0, 2, 1).astype(np.float32))
